# revision 23
# baseline (speedup 1.0000x reference)
"""Trainium2 Bass kernel for nn_ClusterMemory_47923245088802.

Computes: loss = mean_b( logsumexp_n(<x_b/||x_b||, f_n>/temp) - <x_b/||x_b||, f_{t_b}>/temp )
with x [4096,1024], f [32768,1024] (rows ~unit norm), t = corrected_targets.

Sharding: features rows split across 8 cores (4096 each, tensor parallel over
num_samples). Each core computes its [4096 x 4096] logit block on the PE array
in fp8-e4m3 DoubleRow mode and reduces it with exp + row-accumulate on the
scalar engine; the host combines the 8 partial sum-exps with a log (the
cross-shard all-reduce of the CE log-sum-exp).

x is L2-normalized on the host and both operands are pre-scaled by 64 to clear
the e4m3 subnormal band (the 1/64^2 is folded into the constant exp scale), so
the device kernel is a pure matmul->exp->accumulate stream: no norm phase, no
per-row scale, no target-dot matmuls (the 4096 target dots are exact host f32).

Layout: ko-parity-major [P, 2, K2=4, 1024] slices. The DoubleRow pair dim gets
a 4096B stride (512B strides measurably slow the PE's dual-stream SBUF reads:
259 vs 215.5 ns per matmul) while every input DMA stays [128 x 8KB] contiguous
(host pre-tiled). The first x/f slices are further split into k2-pair sub-DMAs
issued in consumption order across the sync and gpsimd queues, so the matmul
stream starts ~9us in; the scalar queue carries no input DMAs (it starves once
the ACTIVATEs start).
"""

import numpy as np
import ml_dtypes

B = 4096          # batch
D = 1024          # feature dim (contraction)
NTOT = 32768      # num_samples
TEMP = 0.05
NCORES = 8
NS = NTOT // NCORES   # samples per core
P = 128
KO = D // P           # 8 k-chunks
K2 = KO // 2          # 4 DoubleRow k-chunk pairs
BT = B // P           # 32 batch tiles
NSL = 8               # 512-column slices of x and of f
SCALE = 64.0          # host pre-scale on x and f before e4m3 quantization
ESCALE = 1.0 / (SCALE * SCALE * TEMP)   # exp scale: dot -> logits

_CACHE = {}


def _build_nc():
    from contextlib import ExitStack

    import concourse.bass as bass
    import concourse.bacc as bacc
    import concourse.mybir as mybir
    import concourse.tile as tile

    f32 = mybir.dt.float32
    fp8 = mybir.dt.float8e4
    AF = mybir.ActivationFunctionType
    DR = mybir.MatmulPerfMode.DoubleRow

    nc = bacc.Bacc("TRN2", target_bir_lowering=False, debug=False,
                   enable_asserts=False)

    xt = nc.dram_tensor("xt", [NSL, P, 2, K2, 512], fp8, kind="ExternalInput")
    ft = nc.dram_tensor("ft", [NSL, P, 2, K2, 512], fp8, kind="ExternalInput")
    # cols 0..31: per-tile accums; 32: tile-31 h1 halves; 33..40: h1 halves
    # of tiles 0..7 (jj=0); 41: pad
    sacc_out = nc.dram_tensor("sacc", [P, BT + 10, 2], f32, kind="ExternalOutput")

    with tile.TileContext(nc) as tc, ExitStack() as ctx:
        io = ctx.enter_context(tc.tile_pool(name="io", bufs=1))
        stats = ctx.enter_context(tc.tile_pool(name="stats", bufs=1))

        x_sb = [io.tile([P, 2, K2, 512], fp8, name=f"xs{j}") for j in range(NSL)]
        f_sb = [io.tile([P, 2, K2, 512], fp8, name=f"fs{j}") for j in range(NSL)]

        # Input DMAs in consumption-deadline order across the sync and gpsimd
        # queues (the scalar queue carries none: it starves once ACTIVATEs
        # start). Whole [128 x 8KB]-contiguous slices only: 8KB descriptors
        # run ~4x faster through a queue than the 2KB ones a k2-split would
        # need, so x0+f0 complete EARLIER than any finer-grained schedule.
        # The first 8 tiles run half-width (2-n-slice) groups, so only x0+f0
        # gate the stream start and f1 isn't needed until ~14us after the
        # first matmul.
        # x0 rides the scalar queue: scalar is idle until the first ACTIVATE
        # and x0 completes before ACTs begin (the scalar queue only starves
        # for transfers still pending once ACTs run). f0/f1 ride sync, whose
        # engine feeds two hardware queues; gpsimd's queue starts ~3.4us
        # late, so it gets only late-deadline slices.
        for j, eng in [(0, nc.sync), (1, nc.sync), (2, nc.gpsimd), (3, nc.gpsimd),
                       (4, nc.sync), (5, nc.gpsimd), (6, nc.sync), (7, nc.gpsimd)]:
            if j < 2:
                nc.sync.dma_start(x_sb[j][:], xt.ap()[j])
            eng.dma_start(f_sb[j][:], ft.ap()[j])
        for j in range(2, NSL):
            eng = nc.sync if j % 2 == 0 else nc.gpsimd
            eng.dma_start(x_sb[j][:], xt.ap()[j])

        # Preload the exp table on the scalar engine during the DMA window:
        # a junk 8-element exp forces walrus's ACT_TABLE_LOAD here instead of
        # in front of the first real (on-critical-path) activation.
        junk = stats.tile([P, 8], f32)
        nc.scalar.activation(junk[:], junk[:], AF.Exp, bias=0.0, scale=0.0)

        sacc_all = stats.tile([P, BT + 10, 2], f32)
        dummy = stats.tile([P, 2048], f32)    # unused act main output
        wz = stats.tile([P, 512], fp8)        # zeros for HAM warmup matmuls
        nc.vector.memset(wz[:], 0.0)

        # Main loop: [4096 x 4096] logits in fp8 DoubleRow, exp + row-sum.
        # 4 n-slices share one 4-bank psum tile so a single wide ACTIVATE
        # covers 2048 columns (amortizes the ACT overhead).
        with tc.tile_pool(name="psm", bufs=2, space="PSUM") as psm:
            # HAM warmup: the PE clock-gate defaults to 1.2 GHz and needs
            # ~3.4us of sustained activity to release to 2.4 GHz. The PE is
            # idle waiting for the first DMAs anyway; burn that window on
            # junk matmuls over a zeroed tile.
            pw = psm.tile([P, 4, 512], f32, name="pl")
            for w in range(14):
                nc.tensor.matmul(pw[:, w % 4, :], wz[:, :P], wz[:],
                                 start=True, stop=True)

            def mms(pl, i, j2s, k2s):
                # k2-major: n-slices per k2-pair, so the startup sub-DMAs are
                # consumed one k2-pair at a time instead of all upfront.
                xw = x_sb[i // 4]
                q = P * (i % 4)
                for k2 in k2s:
                    for gi, j in enumerate(j2s):
                        fw = f_sb[j]
                        nc.tensor.matmul(
                            pl[:, gi, :],
                            xw[:, :, k2, q:q + P],
                            fw[:, :, k2, :],
                            start=k2 == 0, stop=k2 == K2 - 1,
                            perf_mode=DR)

            def emit_group(i, jj):
                pl = psm.tile([P, 4, 512], f32, name="pl")
                mms(pl, i, range(4 * jj, 4 * jj + 4), range(K2))
                nc.scalar.activation(dummy[:], pl[:], AF.Exp, bias=0.0,
                                     scale=ESCALE,
                                     accum_out=sacc_all[:, i, jj:jj + 1])

            def emit_half(i, h, col, jj):
                # half-width group: 2 n-slices, 1024-wide ACT
                pl = psm.tile([P, 4, 512], f32, name="pl")
                mms(pl, i, (2 * h, 2 * h + 1), range(K2))
                nc.scalar.activation(dummy[:, :1024], pl[:, :2, :], AF.Exp,
                                     bias=0.0, scale=ESCALE,
                                     accum_out=sacc_all[:, col, jj:jj + 1])

            # First 8 tiles in half-groups: h=0 rounds touch only x0+f0,
            # h=1 rounds only f1, covering the DMA of the rest.
            for i in range(8):
                emit_half(i, 0, i, 0)
            for i in range(8):
                emit_half(i, 1, 34 + i, 0)
            for i in range(8):
                emit_group(i, 1)
            for i in range(8, BT - 2):
                emit_group(i, 0)
                emit_group(i, 1)
            # Last two tiles in half-groups: a 1024-wide ACT keeps up with
            # its 8-matmul group, so only one short ACT rides the
            # end-of-kernel critical path (a 2048-wide one would stall the
            # psum slot rotation AND sit whole on the tail).
            i = BT - 2
            emit_half(i, 0, i, 0)
            emit_half(i, 1, 33, 0)
            emit_half(i, 2, i, 1)
            emit_half(i, 3, 33, 1)
            nc.sync.dma_start(sacc_out.ap()[:, :BT - 1], sacc_all[:, :BT - 1])
            nc.gpsimd.dma_start(sacc_out.ap()[:, 33:], sacc_all[:, 33:])
            i = BT - 1
            emit_half(i, 0, i, 0)
            emit_half(i, 1, 32, 0)
            emit_half(i, 2, i, 1)
            emit_half(i, 3, 32, 1)

        nc.sync.dma_start(sacc_out.ap()[:, BT - 1:33], sacc_all[:, BT - 1:33])

    nc.compile()
    return nc


def _get_nc():
    if "nc" not in _CACHE:
        _CACHE["nc"] = _build_nc()
    return _CACHE["nc"]


def _tile_slices(aT):
    """[D, N] (d-major) -> [NSL, P, 2, K2, 1024] ko-parity-major slices.

    out[s, p, r, k2, b] = aT[(2*k2 + r)*128 + p, 512*s + b]
    """
    n = aT.shape[1]
    a = aT.reshape(K2, 2, P, n // 512, 512)        # [k2, r, p, s, b]
    return np.ascontiguousarray(a.transpose(3, 2, 1, 0, 4))


def _prep(inputs, corrected_targets, features):
    import concourse.mybir as mybir
    fp8 = mybir.dt.np(mybir.dt.float8e4)
    x = np.asarray(inputs, dtype=np.float32)
    f = np.asarray(features, dtype=np.float32)
    ct = np.asarray(corrected_targets).astype(np.int64)

    xn = x / np.linalg.norm(x, axis=1, keepdims=True)
    tdot = np.einsum('bd,bd->b', xn.astype(np.float64),
                     f[ct].astype(np.float64)) / TEMP

    xt = _tile_slices(np.ascontiguousarray((xn * SCALE).T)).astype(fp8)
    fT = np.ascontiguousarray((f * SCALE).T)                  # [D, NTOT]
    in_maps = []
    for c in range(NCORES):
        in_maps.append({
            "xt": xt,
            "ft": _tile_slices(fT[:, c * NS:(c + 1) * NS]).astype(fp8),
        })
    return in_maps, tdot


def _combine(results, tdot):
    S = np.zeros(B, dtype=np.float64)
    for c in range(NCORES):
        # sacc [P, BT+2, 2]: batch b = i*128 + p, summed over the 2 halves.
        # Tile BT-1 was emitted with per-bank ACTs: its 4 partial accums live
        # in the 2 spare columns; column BT-1 itself is unwritten.
        sacc = results[c]["sacc"].astype(np.float64)
        part = sacc[:, :BT].sum(axis=2)          # [P, BT]
        # h1 half-group accums: tiles 0..7 (jj=0) live in cols 34..41 slot 0;
        # tiles 31/30's h1/h3 halves live in cols 32/33 (both jj slots).
        part[:, 0:8] += sacc[:, 34:42, 0]
        part[:, BT - 1] += sacc[:, 32, :].sum(axis=1)
        part[:, BT - 2] += sacc[:, 33, :].sum(axis=1)
        S += part.T.ravel()
    loss = np.mean(np.log(S) - tdot)
    return np.asarray(loss, dtype=np.float32)


def _run(inputs, targets, corrected_targets, features, trace=False, tmpdir=None):
    import time
    from concourse import bass_utils
    nc = _get_nc()
    in_maps, tdot = _prep(inputs, corrected_targets, features)
    last_exc = None
    for attempt in range(3):
        try:
            res = bass_utils.run_bass_kernel_spmd(
                nc, in_maps, core_ids=list(range(NCORES)), trace=trace,
                tmpdir=tmpdir)
            return _combine(res.results, tdot), res
        except Exception as e:  # transient device state (e.g. prior crash)
            last_exc = e
            time.sleep(2.0)
    raise last_exc


def kernel(inputs, targets, corrected_targets, features):
    out, _ = _run(inputs, targets, corrected_targets, features, trace=False)
    return out


# revision 24
# speedup vs baseline: 1.0155x; 1.0155x over previous
"""Trainium2 Bass kernel for nn_ClusterMemory_47923245088802.

Computes: loss = mean_b( logsumexp_n(<x_b/||x_b||, f_n>/temp) - <x_b/||x_b||, f_{t_b}>/temp )
with x [4096,1024], f [32768,1024] (rows ~unit norm), t = corrected_targets.

Sharding: features rows split across 8 cores (4096 each, tensor parallel over
num_samples). Each core computes its [4096 x 4096] logit block on the PE array
in fp8-e4m3 DoubleRow mode and reduces it with exp + row-accumulate on the
scalar engine; the host combines the 8 partial sum-exps with a log (the
cross-shard all-reduce of the CE log-sum-exp).

x is L2-normalized on the host and both operands are pre-scaled by 64 to clear
the e4m3 subnormal band (the 1/64^2 is folded into the constant exp scale), so
the device kernel is a pure matmul->exp->accumulate stream: no norm phase, no
per-row scale, no target-dot matmuls (the 4096 target dots are exact host f32).

Layout: ko-parity-major [P, 2, K2=4, 1024] slices. The DoubleRow pair dim gets
a 4096B stride (512B strides measurably slow the PE's dual-stream SBUF reads:
259 vs 215.5 ns per matmul) while every input DMA stays [128 x 8KB] contiguous
(host pre-tiled). The first x/f slices are further split into k2-pair sub-DMAs
issued in consumption order across the sync and gpsimd queues, so the matmul
stream starts ~9us in; the scalar queue carries no input DMAs (it starves once
the ACTIVATEs start).
"""

import numpy as np
import ml_dtypes

B = 4096          # batch
D = 1024          # feature dim (contraction)
NTOT = 32768      # num_samples
TEMP = 0.05
NCORES = 8
NS = NTOT // NCORES   # samples per core
P = 128
KO = D // P           # 8 k-chunks
K2 = KO // 2          # 4 DoubleRow k-chunk pairs
BT = B // P           # 32 batch tiles
NSL = 4               # 1024-column slices of x and of f
SCALE = 64.0          # host pre-scale on x and f before e4m3 quantization
ESCALE = 1.0 / (SCALE * SCALE * TEMP)   # exp scale: dot -> logits

_CACHE = {}


def _build_nc():
    from contextlib import ExitStack

    import concourse.bass as bass
    import concourse.bacc as bacc
    import concourse.mybir as mybir
    import concourse.tile as tile

    f32 = mybir.dt.float32
    fp8 = mybir.dt.float8e4
    AF = mybir.ActivationFunctionType
    DR = mybir.MatmulPerfMode.DoubleRow

    nc = bacc.Bacc("TRN2", target_bir_lowering=False, debug=False,
                   enable_asserts=False)

    xt = nc.dram_tensor("xt", [NSL, P, 2, K2, 1024], fp8, kind="ExternalInput")
    ft = nc.dram_tensor("ft", [NSL, P, 2, K2, 1024], fp8, kind="ExternalInput")
    # cols 0..31: per-tile accums; 32: tile-31 h1 halves; 33..40: h1 halves
    # of tiles 0..7 (jj=0); 41: pad
    sacc_out = nc.dram_tensor("sacc", [P, BT + 10, 2], f32, kind="ExternalOutput")

    with tile.TileContext(nc) as tc, ExitStack() as ctx:
        io = ctx.enter_context(tc.tile_pool(name="io", bufs=1))
        stats = ctx.enter_context(tc.tile_pool(name="stats", bufs=1))

        x_sb = [io.tile([P, 2, K2, 1024], fp8, name=f"xs{j}") for j in range(NSL)]
        f_sb = [io.tile([P, 2, K2, 1024], fp8, name=f"fs{j}") for j in range(NSL)]

        # Input DMAs in consumption-deadline order across the sync and gpsimd
        # queues (the scalar queue carries none: it starves once ACTIVATEs
        # start). Whole [128 x 8KB]-contiguous slices only: 8KB descriptors
        # run ~4x faster through a queue than the 2KB ones a k2-split would
        # need, so x0+f0 complete EARLIER than any finer-grained schedule.
        # The first 8 tiles run half-width (2-n-slice) groups, so only x0+f0
        # gate the stream start and f1 isn't needed until ~14us after the
        # first matmul.
        # x0 rides the scalar queue: scalar is idle until the first ACTIVATE
        # and x0 completes before ACTs begin (the scalar queue only starves
        # for transfers still pending once ACTs run). f0/f1 ride sync, whose
        # engine feeds two hardware queues; gpsimd's queue starts ~3.4us
        # late, so it gets only late-deadline slices.
        nc.scalar.dma_start(x_sb[0][:], xt.ap()[0])
        nc.sync.dma_start(f_sb[0][:], ft.ap()[0])
        nc.sync.dma_start(f_sb[1][:], ft.ap()[1])
        nc.gpsimd.dma_start(f_sb[2][:], ft.ap()[2])
        nc.gpsimd.dma_start(f_sb[3][:], ft.ap()[3])
        nc.sync.dma_start(x_sb[1][:], xt.ap()[1])
        nc.gpsimd.dma_start(x_sb[2][:], xt.ap()[2])
        nc.sync.dma_start(x_sb[3][:], xt.ap()[3])

        # Preload the exp table on the scalar engine during the DMA window:
        # a junk 8-element exp forces walrus's ACT_TABLE_LOAD here instead of
        # in front of the first real (on-critical-path) activation.
        junk = stats.tile([P, 8], f32)
        nc.scalar.activation(junk[:], junk[:], AF.Exp, bias=0.0, scale=0.0)

        sacc_all = stats.tile([P, BT + 10, 2], f32)
        dummy = stats.tile([P, 2048], f32)    # unused act main output
        wz = stats.tile([P, 512], fp8)        # zeros for HAM warmup matmuls
        nc.vector.memset(wz[:], 0.0)

        # Main loop: [4096 x 4096] logits in fp8 DoubleRow, exp + row-sum.
        # 4 n-slices share one 4-bank psum tile so a single wide ACTIVATE
        # covers 2048 columns (amortizes the ACT overhead).
        with tc.tile_pool(name="psm", bufs=2, space="PSUM") as psm:
            # HAM warmup: the PE clock-gate defaults to 1.2 GHz and needs
            # ~3.4us of sustained activity to release to 2.4 GHz. The PE is
            # idle waiting for the first DMAs anyway; burn that window on
            # junk matmuls over a zeroed tile.
            pw = psm.tile([P, 4, 512], f32, name="pl")
            for w in range(14):
                nc.tensor.matmul(pw[:, w % 4, :], wz[:, :P], wz[:],
                                 start=True, stop=True)

            def mms(pl, i, j2s, k2s):
                # k2-major: n-slices per k2-pair, so the startup sub-DMAs are
                # consumed one k2-pair at a time instead of all upfront.
                xw = x_sb[i // 8]
                q = P * (i % 8)
                for k2 in k2s:
                    for gi, j in enumerate(j2s):
                        fw = f_sb[j // 2]
                        nf = 512 * (j % 2)
                        nc.tensor.matmul(
                            pl[:, gi, :],
                            xw[:, :, k2, q:q + P],
                            fw[:, :, k2, nf:nf + 512],
                            start=k2 == 0, stop=k2 == K2 - 1,
                            perf_mode=DR)

            def emit_group(i, jj):
                pl = psm.tile([P, 4, 512], f32, name="pl")
                mms(pl, i, range(4 * jj, 4 * jj + 4), range(K2))
                nc.scalar.activation(dummy[:], pl[:], AF.Exp, bias=0.0,
                                     scale=ESCALE,
                                     accum_out=sacc_all[:, i, jj:jj + 1])

            def emit_half(i, h, col, jj):
                # half-width group: 2 n-slices, 1024-wide ACT
                pl = psm.tile([P, 4, 512], f32, name="pl")
                mms(pl, i, (2 * h, 2 * h + 1), range(K2))
                nc.scalar.activation(dummy[:, :1024], pl[:, :2, :], AF.Exp,
                                     bias=0.0, scale=ESCALE,
                                     accum_out=sacc_all[:, col, jj:jj + 1])

            # First 8 tiles in half-groups: h=0 rounds touch only x0+f0,
            # h=1 rounds only f1, covering the DMA of the rest.
            for i in range(8):
                emit_half(i, 0, i, 0)
            for i in range(8):
                emit_half(i, 1, 34 + i, 0)
            for i in range(8):
                emit_group(i, 1)
            for i in range(8, BT - 2):
                emit_group(i, 0)
                emit_group(i, 1)
            # Last two tiles in half-groups: a 1024-wide ACT keeps up with
            # its 8-matmul group, so only one short ACT rides the
            # end-of-kernel critical path (a 2048-wide one would stall the
            # psum slot rotation AND sit whole on the tail).
            i = BT - 2
            emit_half(i, 0, i, 0)
            emit_half(i, 1, 33, 0)
            emit_half(i, 2, i, 1)
            emit_half(i, 3, 33, 1)
            nc.sync.dma_start(sacc_out.ap()[:, :BT - 1], sacc_all[:, :BT - 1])
            nc.gpsimd.dma_start(sacc_out.ap()[:, 33:], sacc_all[:, 33:])
            i = BT - 1
            emit_half(i, 0, i, 0)
            emit_half(i, 1, 32, 0)
            emit_half(i, 2, i, 1)
            emit_half(i, 3, 32, 1)

        nc.sync.dma_start(sacc_out.ap()[:, BT - 1:33], sacc_all[:, BT - 1:33])

    nc.compile()
    return nc


def _get_nc():
    if "nc" not in _CACHE:
        _CACHE["nc"] = _build_nc()
    return _CACHE["nc"]


def _tile_slices(aT):
    """[D, N] (d-major) -> [NSL, P, 2, K2, 1024] ko-parity-major slices.

    out[s, p, r, k2, b] = aT[(2*k2 + r)*128 + p, 1024*s + b]
    """
    n = aT.shape[1]
    a = aT.reshape(K2, 2, P, n // 1024, 1024)      # [k2, r, p, s, b]
    return np.ascontiguousarray(a.transpose(3, 2, 1, 0, 4))


def _prep(inputs, corrected_targets, features):
    import concourse.mybir as mybir
    fp8 = mybir.dt.np(mybir.dt.float8e4)
    x = np.asarray(inputs, dtype=np.float32)
    f = np.asarray(features, dtype=np.float32)
    ct = np.asarray(corrected_targets).astype(np.int64)

    xn = x / np.linalg.norm(x, axis=1, keepdims=True)
    tdot = np.einsum('bd,bd->b', xn.astype(np.float64),
                     f[ct].astype(np.float64)) / TEMP

    xt = _tile_slices(np.ascontiguousarray((xn * SCALE).T)).astype(fp8)
    fT = np.ascontiguousarray((f * SCALE).T)                  # [D, NTOT]
    in_maps = []
    for c in range(NCORES):
        in_maps.append({
            "xt": xt,
            "ft": _tile_slices(fT[:, c * NS:(c + 1) * NS]).astype(fp8),
        })
    return in_maps, tdot


def _combine(results, tdot):
    S = np.zeros(B, dtype=np.float64)
    for c in range(NCORES):
        # sacc [P, BT+2, 2]: batch b = i*128 + p, summed over the 2 halves.
        # Tile BT-1 was emitted with per-bank ACTs: its 4 partial accums live
        # in the 2 spare columns; column BT-1 itself is unwritten.
        sacc = results[c]["sacc"].astype(np.float64)
        part = sacc[:, :BT].sum(axis=2)          # [P, BT]
        # h1 half-group accums: tiles 0..7 (jj=0) live in cols 34..41 slot 0;
        # tiles 31/30's h1/h3 halves live in cols 32/33 (both jj slots).
        part[:, 0:8] += sacc[:, 34:42, 0]
        part[:, BT - 1] += sacc[:, 32, :].sum(axis=1)
        part[:, BT - 2] += sacc[:, 33, :].sum(axis=1)
        S += part.T.ravel()
    loss = np.mean(np.log(S) - tdot)
    return np.asarray(loss, dtype=np.float32)


def _run(inputs, targets, corrected_targets, features, trace=False, tmpdir=None):
    import time
    from concourse import bass_utils
    nc = _get_nc()
    in_maps, tdot = _prep(inputs, corrected_targets, features)
    last_exc = None
    for attempt in range(3):
        try:
            res = bass_utils.run_bass_kernel_spmd(
                nc, in_maps, core_ids=list(range(NCORES)), trace=trace,
                tmpdir=tmpdir)
            return _combine(res.results, tdot), res
        except Exception as e:  # transient device state (e.g. prior crash)
            last_exc = e
            time.sleep(2.0)
    raise last_exc


def kernel(inputs, targets, corrected_targets, features):
    out, _ = _run(inputs, targets, corrected_targets, features, trace=False)
    return out


# revision 27
# speedup vs baseline: 1.0360x; 1.0202x over previous
"""Trainium2 Bass kernel for nn_ClusterMemory_47923245088802.

Computes: loss = mean_b( logsumexp_n(<x_b/||x_b||, f_n>/temp) - <x_b/||x_b||, f_{t_b}>/temp )
with x [4096,1024], f [32768,1024] (rows ~unit norm), t = corrected_targets.

Sharding: features rows split across 8 cores (4096 each, tensor parallel over
num_samples). Each core computes its [4096 x 4096] logit block on the PE array
in fp8-e4m3 DoubleRow mode and reduces it with exp + row-accumulate on the
scalar engine; the host combines the 8 partial sum-exps with a log (the
cross-shard all-reduce of the CE log-sum-exp).

x is L2-normalized on the host and both operands are pre-scaled by 64 to clear
the e4m3 subnormal band (the 1/64^2 is folded into the constant exp scale), so
the device kernel is a pure matmul->exp->accumulate stream: no norm phase, no
per-row scale, no target-dot matmuls (the 4096 target dots are exact host f32).

Layout: ko-parity-major [P, 2, K2=4, 512] slices. The DoubleRow pair dim gets
a 2048B stride (512B strides slow the PE's dual-stream SBUF reads to 259 vs
215.5 ns per matmul; 2048B and 4096B measured full-speed) and every input DMA
is a whole [128 x 4KB]-contiguous slice (host pre-tiled; big descriptors move
much faster through a queue than sub-2KB ones). Slices are issued in
consumption-deadline order on the sync and gpsimd queues; quarter-width groups
for the first 4 batch tiles mean only x0+f0 (1MB) gate the stream start
(~12us), and half-width groups for the last 2 tiles keep the final ACT short.
~5us of junk warmup matmuls hold the PE's HAM clock-gate open until data
lands. Matmul stream measures 215.5 ns/matmul (the N=512 fp8-DR issue floor).
Occasionally a run lands ~20% slower wholesale (P0 power-state downclock to
2.0 GHz) — rerun, don't chase phantom regressions.
"""

import numpy as np
import ml_dtypes

B = 4096          # batch
D = 1024          # feature dim (contraction)
NTOT = 32768      # num_samples
TEMP = 0.05
NCORES = 8
NS = NTOT // NCORES   # samples per core
P = 128
KO = D // P           # 8 k-chunks
K2 = KO // 2          # 4 DoubleRow k-chunk pairs
BT = B // P           # 32 batch tiles
NSL = 8               # 512-column slices of x and of f
SCALE = 64.0          # host pre-scale on x and f before e4m3 quantization
ESCALE = 1.0 / (SCALE * SCALE * TEMP)   # exp scale: dot -> logits

_CACHE = {}


def _build_nc():
    from contextlib import ExitStack

    import concourse.bass as bass
    import concourse.bacc as bacc
    import concourse.mybir as mybir
    import concourse.tile as tile

    f32 = mybir.dt.float32
    fp8 = mybir.dt.float8e4
    AF = mybir.ActivationFunctionType
    DR = mybir.MatmulPerfMode.DoubleRow

    nc = bacc.Bacc("TRN2", target_bir_lowering=False, debug=False,
                   enable_asserts=False)

    xt = nc.dram_tensor("xt", [NSL, P, 2, K2, 512], fp8, kind="ExternalInput")
    ft = nc.dram_tensor("ft", [NSL, P, 2, K2, 512], fp8, kind="ExternalInput")
    # accum columns: 0..31 per-tile [i, jj]; 32: tile-31 extras; 33: tile-30
    # extras; 34..37: tiles 0-3 quarter j=1; 38..41: tiles 0-3 half h1;
    # 42..45: tiles 4-7 half h1.
    sacc_out = nc.dram_tensor("sacc", [P, BT + 14, 2], f32, kind="ExternalOutput")

    with tile.TileContext(nc) as tc, ExitStack() as ctx:
        io = ctx.enter_context(tc.tile_pool(name="io", bufs=1))
        stats = ctx.enter_context(tc.tile_pool(name="stats", bufs=1))

        x_sb = [io.tile([P, 2, K2, 512], fp8, name=f"xs{j}") for j in range(NSL)]
        f_sb = [io.tile([P, 2, K2, 512], fp8, name=f"fs{j}") for j in range(NSL)]

        # Input DMAs in consumption-deadline order. x0+f0 first (they alone
        # gate the quarter-group stream start) serial on sync, whose queue
        # starts ~3us before gpsimd's and bursts well above fair-share while
        # alone; the scalar queue carries nothing (it starves once ACTIVATEs
        # run).
        nc.sync.dma_start(x_sb[0][:], xt.ap()[0])
        nc.sync.dma_start(f_sb[0][:], ft.ap()[0])
        nc.sync.dma_start(f_sb[1][:], ft.ap()[1])
        nc.gpsimd.dma_start(f_sb[2][:], ft.ap()[2])
        nc.sync.dma_start(x_sb[1][:], xt.ap()[1])
        nc.gpsimd.dma_start(f_sb[3][:], ft.ap()[3])
        nc.sync.dma_start(f_sb[5][:], ft.ap()[5])
        nc.gpsimd.dma_start(f_sb[4][:], ft.ap()[4])
        nc.sync.dma_start(f_sb[7][:], ft.ap()[7])
        nc.gpsimd.dma_start(f_sb[6][:], ft.ap()[6])
        nc.sync.dma_start(x_sb[3][:], xt.ap()[3])
        nc.gpsimd.dma_start(x_sb[2][:], xt.ap()[2])
        nc.sync.dma_start(x_sb[5][:], xt.ap()[5])
        nc.gpsimd.dma_start(x_sb[4][:], xt.ap()[4])
        nc.sync.dma_start(x_sb[7][:], xt.ap()[7])
        nc.gpsimd.dma_start(x_sb[6][:], xt.ap()[6])

        # Preload the exp table on the scalar engine during the DMA window:
        # a junk 8-element exp forces walrus's ACT_TABLE_LOAD here instead of
        # in front of the first real (on-critical-path) activation.
        junk = stats.tile([P, 8], f32)
        nc.scalar.activation(junk[:], junk[:], AF.Exp, bias=0.0, scale=0.0)

        sacc_all = stats.tile([P, BT + 14, 2], f32)
        dummy = stats.tile([P, 2048], f32)    # unused act main output
        wz = stats.tile([P, 512], fp8)        # zeros for HAM warmup matmuls
        nc.vector.memset(wz[:], 0.0)

        # Main loop: [4096 x 4096] logits in fp8 DoubleRow, exp + row-sum.
        # Steady state: 4 n-slices share one 4-bank psum tile so a single
        # wide ACTIVATE covers 2048 columns (amortizes the ACT overhead).
        with tc.tile_pool(name="psm", bufs=2, space="PSUM") as psm:
            # HAM warmup: the PE clock-gate defaults to 1.2 GHz and needs
            # ~3.4us of sustained activity to release to 2.4 GHz. The PE is
            # idle waiting for the first DMAs anyway; burn that window on
            # junk matmuls over a zeroed tile.
            pw = psm.tile([P, 4, 512], f32, name="pl")
            for w in range(12):
                nc.tensor.matmul(pw[:, w % 4, :], wz[:, :P], wz[:],
                                 start=True, stop=True)

            def mms(pl, i, j2s):
                xw = x_sb[i // 4]
                q = P * (i % 4)
                # k2-major: weight reuse across the n-slices of the group
                for k2 in range(K2):
                    for gi, j in enumerate(j2s):
                        nc.tensor.matmul(
                            pl[:, gi, :],
                            xw[:, :, k2, q:q + P],
                            f_sb[j][:, :, k2, :],
                            start=k2 == 0, stop=k2 == K2 - 1,
                            perf_mode=DR)

            def emit(i, j2s, col, jj):
                pl = psm.tile([P, 4, 512], f32, name="pl")
                mms(pl, i, j2s)
                n = 512 * len(j2s)
                nc.scalar.activation(dummy[:, :n], pl[:, :len(j2s), :],
                                     AF.Exp, bias=0.0, scale=ESCALE,
                                     accum_out=sacc_all[:, col, jj:jj + 1])

            # Quarter-groups for tiles 0..3: only x0+f0 (1MB) gate the
            # stream start; f1 isn't needed until ~3.5us later.
            for i in range(4):
                emit(i, (0,), i, 0)
            for i in range(4):
                emit(i, (1,), 34 + i, 0)
            # Half-groups while the DMA front catches up.
            for i in range(4, 8):
                emit(i, (0, 1), i, 0)
            for i in range(4):
                emit(i, (2, 3), 38 + i, 0)
            for i in range(4, 8):
                emit(i, (2, 3), 42 + i - 4, 0)
            # Steady state: full-width groups.
            for i in range(8):
                emit(i, (4, 5, 6, 7), i, 1)
            for i in range(8, BT - 2):
                emit(i, (0, 1, 2, 3), i, 0)
                emit(i, (4, 5, 6, 7), i, 1)
            # Last two tiles in half-groups: a 1024-wide ACT keeps up with
            # its 8-matmul group, so only one short ACT rides the
            # end-of-kernel critical path.
            i = BT - 2
            emit(i, (0, 1), i, 0)
            emit(i, (2, 3), 33, 0)
            emit(i, (4, 5), i, 1)
            emit(i, (6, 7), 33, 1)
            nc.sync.dma_start(sacc_out.ap()[:, :BT - 1], sacc_all[:, :BT - 1])
            nc.gpsimd.dma_start(sacc_out.ap()[:, 33:], sacc_all[:, 33:])
            i = BT - 1
            emit(i, (0, 1), i, 0)
            emit(i, (2, 3), 32, 0)
            emit(i, (4, 5), i, 1)
            emit(i, (6, 7), 32, 1)

        nc.sync.dma_start(sacc_out.ap()[:, BT - 1:33], sacc_all[:, BT - 1:33])

    nc.compile()
    return nc


def _get_nc():
    if "nc" not in _CACHE:
        _CACHE["nc"] = _build_nc()
    return _CACHE["nc"]


def _tile_slices(aT):
    """[D, N] (d-major) -> [NSL, P, 2, K2, 512] ko-parity-major slices.

    out[s, p, r, k2, b] = aT[(2*k2 + r)*128 + p, 512*s + b]
    """
    n = aT.shape[1]
    a = aT.reshape(K2, 2, P, n // 512, 512)        # [k2, r, p, s, b]
    return np.ascontiguousarray(a.transpose(3, 2, 1, 0, 4))


def _prep(inputs, corrected_targets, features):
    import concourse.mybir as mybir
    fp8 = mybir.dt.np(mybir.dt.float8e4)
    x = np.asarray(inputs, dtype=np.float32)
    f = np.asarray(features, dtype=np.float32)
    ct = np.asarray(corrected_targets).astype(np.int64)

    xn = x / np.linalg.norm(x, axis=1, keepdims=True)
    tdot = np.einsum('bd,bd->b', xn.astype(np.float64),
                     f[ct].astype(np.float64)) / TEMP

    xt = _tile_slices(np.ascontiguousarray((xn * SCALE).T)).astype(fp8)
    fT = np.ascontiguousarray((f * SCALE).T)                  # [D, NTOT]
    in_maps = []
    for c in range(NCORES):
        in_maps.append({
            "xt": xt,
            "ft": _tile_slices(fT[:, c * NS:(c + 1) * NS]).astype(fp8),
        })
    return in_maps, tdot


def _combine(results, tdot):
    S = np.zeros(B, dtype=np.float64)
    for c in range(NCORES):
        sacc = results[c]["sacc"].astype(np.float64)
        part = sacc[:, :BT].sum(axis=2)          # [P, BT]
        # partial-group accums (all jj=0 slots of their extra columns):
        # tiles 0-3: quarter j=1 in 34..37, half h1 in 38..41;
        # tiles 4-7: half h1 in 42..45; tiles 30/31 extras in 33/32 (both jj).
        part[:, 0:4] += sacc[:, 34:38, 0] + sacc[:, 38:42, 0]
        part[:, 4:8] += sacc[:, 42:46, 0]
        part[:, BT - 2] += sacc[:, 33, :].sum(axis=1)
        part[:, BT - 1] += sacc[:, 32, :].sum(axis=1)
        S += part.T.ravel()
    loss = np.mean(np.log(S) - tdot)
    return np.asarray(loss, dtype=np.float32)


def _run(inputs, targets, corrected_targets, features, trace=False, tmpdir=None):
    import time
    from concourse import bass_utils
    nc = _get_nc()
    in_maps, tdot = _prep(inputs, corrected_targets, features)
    last_exc = None
    for attempt in range(3):
        try:
            res = bass_utils.run_bass_kernel_spmd(
                nc, in_maps, core_ids=list(range(NCORES)), trace=trace,
                tmpdir=tmpdir)
            return _combine(res.results, tdot), res
        except Exception as e:  # transient device state (e.g. prior crash)
            last_exc = e
            time.sleep(2.0)
    raise last_exc


def kernel(inputs, targets, corrected_targets, features):
    out, _ = _run(inputs, targets, corrected_targets, features, trace=False)
    return out
